# revision 8
# baseline (speedup 1.0000x reference)
"""MoE regressor Trainium2 kernel.

Data-parallel across 8 NeuronCores: each core handles 2048 of the 16384
tokens and runs the full model (gating + all 8 experts, dense) on its slice.

Per-core layout (two passes over 1024-token halves):
  - x tile rows are PE-transposed; the psum tiles are copied into an f32
    staging tile (feeds exact fp32 gating matmuls), then converted once to
    an fp32r-resident xT (d on partitions) for the expert matmuls.
  - logits = x @ Wg + bg via fp32 matmuls (exact-ish; feeds top-k decisions).
  - top-2 via DVE max / max_index (full top-8 sort), softmax via ACT Exp.
  - expert matmuls in fp32r (full PE rate at N>=256), K accumulated in PSUM,
    combined into the SBUF accumulator with one fused DVE op per tile:
      acc = psum * w[:, e] + acc   (scalar_tensor_tensor)
  - be contribution initializes acc via a K=8 fp32 matmul with sw^T.

Hardware constraint worked around throughout: fp32/fp32r/transpose matmuls
lower through the S3_LW struct which supports only ONE semaphore wait. So
every tensor a PE instruction reads is produced by a DVE instruction (DMA
and host data are laundered through DVE copies), and every PSUM consumer is
DVE — all PE waits then coalesce onto the single DVE semaphore.
"""

import numpy as np

import concourse.bass as bass
import concourse.mybir as mybir
from concourse import bacc
from concourse import bass_utils
from concourse.tile import TileContext

F32 = mybir.dt.float32
F32R = mybir.dt.float32r
I32 = mybir.dt.int32
U32 = mybir.dt.uint32
AX = mybir.AxisListType
OP = mybir.AluOpType
ACTF = mybir.ActivationFunctionType

N_CORES = 8
B = 16384
D = 2048
O = 1024
E = 8
B_L = B // N_CORES          # tokens per core
HALF = 1024                 # tokens per pass
N_PASS = B_L // HALF
TT_P = HALF // 128          # token tiles per pass (8)
KT = D // 128               # contraction tiles (16)
ON = 256                    # output chunk (matmul free dim)
OQ = O // ON                # output chunks (4)
XL_CH = 512                 # x load chunk (d columns per load)
WE_CH = 8                   # we load chunk (kt tiles per load)


def build_nc():
    nc = bacc.Bacc("TRN2")

    x_d = nc.dram_tensor("x", [B_L, D], F32, kind="ExternalInput")
    wg_d = nc.dram_tensor("Wg", [D, E], F32, kind="ExternalInput")
    we_d = nc.dram_tensor("We", [E, D, O], F32, kind="ExternalInput")
    be_d = nc.dram_tensor("be", [E, O], F32, kind="ExternalInput")
    # host-prepared constants
    id_d = nc.dram_tensor("identity", [128, 128], F32, kind="ExternalInput")
    bgb_d = nc.dram_tensor("bg_bcast", [128, E], F32, kind="ExternalInput")

    pred_d = nc.dram_tensor("pred", [B_L, O], F32, kind="ExternalOutput")
    logits_d = nc.dram_tensor("logits", [B_L, E], F32, kind="ExternalOutput")
    sw_d = nc.dram_tensor("sparse_weights", [B_L, E], F32, kind="ExternalOutput")
    ti_d = nc.dram_tensor("top_indices", [B_L, 2], I32, kind="ExternalOutput")
    tv_d = nc.dram_tensor("top_values", [B_L, 2], F32, kind="ExternalOutput")

    with TileContext(nc) as tc:
        with (
            tc.tile_pool(name="const", bufs=1) as const_pool,
            tc.tile_pool(name="xraw", bufs=2) as xraw_pool,
            tc.tile_pool(name="xload", bufs=2) as xload_pool,
            tc.tile_pool(name="stage", bufs=1) as stage_pool,
            tc.tile_pool(name="xt", bufs=1) as xt_pool,
            tc.tile_pool(name="acc", bufs=1) as acc_pool,
            tc.tile_pool(name="weraw", bufs=2) as weraw_pool,
            tc.tile_pool(name="we", bufs=2) as we_pool,
            tc.tile_pool(name="gat", bufs=2) as gat_pool,
            tc.tile_pool(name="sw", bufs=2) as sw_pool,
            tc.tile_pool(name="tp_ps", bufs=2, space="PSUM") as tp_psum,
            tc.tile_pool(name="lg_ps", bufs=1, space="PSUM") as lg_psum,
            tc.tile_pool(name="st_ps", bufs=1, space="PSUM") as st_psum,
            tc.tile_pool(name="mm_ps", bufs=4, space="PSUM") as mm_psum,
        ):
            # ---- constants: DMA in, then launder through DVE for PE use ----
            id_raw = const_pool.tile([128, 128], F32)
            nc.sync.dma_start(id_raw[:, :], id_d[:, :])
            identity = const_pool.tile([128, 128], F32)
            nc.vector.tensor_copy(identity[:, :], id_raw[:, :])

            wg_raw = const_pool.tile([128, KT, E], F32)
            nc.sync.dma_start(wg_raw[:, :, :], wg_d.rearrange("(kt p) e -> p kt e", p=128))
            wg_sb = const_pool.tile([128, KT, E], F32)
            nc.vector.tensor_copy(wg_sb[:, :, :], wg_raw[:, :, :])

            be_raw = const_pool.tile([E, O], F32)
            nc.sync.dma_start(be_raw[:, :], be_d[:, :])
            be_sb = const_pool.tile([E, O], F32)
            nc.vector.tensor_copy(be_sb[:, :], be_raw[:, :])

            bgb_sb = const_pool.tile([128, E], F32)
            nc.sync.dma_start(bgb_sb[:, :], bgb_d[:, :])
            # DVE observes the bg DMA early so the per-tile logit adds carry
            # only the PE wait
            dve_dum = const_pool.tile([1, E], F32)
            nc.vector.tensor_copy(dve_dum[:, :], bgb_sb[0:1, :])

            for h in range(N_PASS):
                t0 = h * HALF  # first token of this pass

                # ---- stage 0+1: load, transpose, gate ----
                xt_tiles = []
                for tt in range(TT_P):
                    xt_tiles.append(xt_pool.tile([128, KT, 128], F32R, tag=f"xt{tt}", name=f"xt_{h}_{tt}"))

                sw_tiles = []
                swt_tiles = []
                for tt in range(TT_P):
                    rows = slice(t0 + tt * 128, t0 + (tt + 1) * 128)

                    stage = stage_pool.tile([128, KT, 128], F32, tag="stage")
                    for xh in range(D // XL_CH):
                        xraw = xraw_pool.tile([128, XL_CH], F32, tag="xraw")
                        nc.sync.dma_start(
                            xraw[:, :], x_d[rows, xh * XL_CH : (xh + 1) * XL_CH]
                        )
                        xl = xload_pool.tile([128, XL_CH], F32, tag="xl")
                        nc.vector.tensor_copy(xl[:, :], xraw[:, :])
                        for kk in range(XL_CH // 128):
                            kt = xh * (XL_CH // 128) + kk
                            tp = tp_psum.tile([128, 128], F32, tag="tp")
                            nc.tensor.transpose(tp[:, :], xl[:, kk * 128 : (kk + 1) * 128], identity[:, :])
                            nc.vector.tensor_copy(stage[:, kt, :], tp[:, :])

                    # convert the full staged [d, tok] block to fp32r once (DVE)
                    nc.vector.tensor_copy(xt_tiles[tt][:, :, :], stage[:, :, :])

                    # gating matmuls in fp32 from the staging tile
                    lg = lg_psum.tile([128, E], F32, tag="lg")
                    for kt in range(KT):
                        nc.tensor.matmul(
                            lg[:, :], stage[:, kt, :], wg_sb[:, kt, :],
                            start=(kt == 0), stop=(kt == KT - 1),
                        )

                    lsb = gat_pool.tile([128, E], F32, tag="lsb")
                    nc.vector.tensor_add(lsb[:, :], lg[:, :], bgb_sb[:, :])
                    nc.sync.dma_start(logits_d[rows, :], lsb[:, :])

                    ls = gat_pool.tile([128, E], F32, tag="ls")  # sorted desc
                    nc.vector.max(out=ls[:, :], in_=lsb[:, :])
                    idx = gat_pool.tile([128, E], U32, tag="idx")
                    nc.vector.max_index(idx[:, :], ls[:, :], lsb[:, :])
                    ti_sb = gat_pool.tile([128, 2], I32, tag="ti")
                    nc.vector.tensor_copy(ti_sb[:, :], idx[:, 0:2])
                    nc.sync.dma_start(ti_d[rows, :], ti_sb[:, :])

                    negm = gat_pool.tile([128, 1], F32, tag="negm")
                    nc.vector.tensor_scalar_mul(negm[:, :], ls[:, 0:1], -1.0)
                    e_all = gat_pool.tile([128, E], F32, tag="eall")
                    nc.scalar.activation(e_all[:, :], lsb[:, :], ACTF.Exp, bias=negm[:, :])
                    e_srt = gat_pool.tile([128, E], F32, tag="esrt")
                    nc.scalar.activation(e_srt[:, :], ls[:, :], ACTF.Exp, bias=negm[:, :])

                    ssum = gat_pool.tile([128, 1], F32, tag="ssum")
                    nc.vector.tensor_reduce(ssum[:, :], e_all[:, :], axis=AX.X, op=OP.add)
                    rsum = gat_pool.tile([128, 1], F32, tag="rsum")
                    nc.vector.reciprocal(rsum[:, :], ssum[:, :])

                    tv_sb = gat_pool.tile([128, 2], F32, tag="tv")
                    nc.vector.tensor_scalar(tv_sb[:, :], e_srt[:, 0:2], rsum[:, :], None, op0=OP.mult)
                    nc.sync.dma_start(tv_d[rows, :], tv_sb[:, :])

                    # normalizer: 1 / (p1 + p2 + 1e-8), as probs
                    d0 = gat_pool.tile([128, 1], F32, tag="d0")
                    nc.vector.tensor_add(d0[:, :], e_srt[:, 0:1], e_srt[:, 1:2])
                    pd = gat_pool.tile([128, 1], F32, tag="pd")
                    nc.vector.tensor_scalar(pd[:, :], d0[:, :], rsum[:, :], 1e-8, op0=OP.mult, op1=OP.add)
                    rden = gat_pool.tile([128, 1], F32, tag="rden")
                    nc.vector.reciprocal(rden[:, :], pd[:, :])
                    fac = gat_pool.tile([128, 1], F32, tag="fac")
                    nc.vector.tensor_mul(fac[:, :], rsum[:, :], rden[:, :])

                    mask = gat_pool.tile([128, E], F32, tag="mask")
                    nc.vector.tensor_scalar(mask[:, :], lsb[:, :], ls[:, 1:2], None, op0=OP.is_ge)
                    swu = gat_pool.tile([128, E], F32, tag="swu")
                    nc.vector.tensor_mul(swu[:, :], e_all[:, :], mask[:, :])
                    sw_sb = sw_pool.tile([128, E], F32, tag=f"sw{tt}")
                    nc.vector.tensor_scalar(sw_sb[:, :], swu[:, :], fac[:, :], None, op0=OP.mult)
                    nc.sync.dma_start(sw_d[rows, :], sw_sb[:, :])
                    sw_tiles.append(sw_sb)

                    # sw^T for the be-init matmul
                    st_ps = st_psum.tile([E, 128], F32, tag="stp")
                    nc.tensor.transpose(st_ps[:, :], sw_sb[:, :], identity[:, :])
                    swt_sb = sw_pool.tile([E, 128], F32, tag=f"swt{tt}")
                    nc.vector.tensor_copy(swt_sb[:, :], st_ps[:, :])
                    swt_tiles.append(swt_sb)

                # ---- stage 1.5: init acc with bias contribution sum_e w[b,e] be[e,:] ----
                acc_tiles = []
                for tt in range(TT_P):
                    acc_tiles.append(acc_pool.tile([128, O], F32, tag=f"acc{tt}", name=f"acc_{h}_{tt}"))
                for tt in range(TT_P):
                    for oq in range(OQ):
                        osl = slice(oq * ON, (oq + 1) * ON)
                        bp = mm_psum.tile([128, ON], F32, tag="mmps")
                        nc.tensor.matmul(
                            bp[:, :], swt_tiles[tt][:, :], be_sb[:, osl],
                            start=True, stop=True,
                        )
                        nc.vector.tensor_copy(acc_tiles[tt][:, osl], bp[:, :])

                # ---- stage 2: experts ----
                for e in range(E):
                    for oq in range(OQ):
                        osl = slice(oq * ON, (oq + 1) * ON)
                        we_t = we_pool.tile([128, KT, ON], F32R, tag="we")
                        for wh in range(KT // WE_CH):
                            ksl = slice(wh * WE_CH, (wh + 1) * WE_CH)
                            we_raw = weraw_pool.tile([128, WE_CH, ON], F32, tag="weraw")
                            nc.sync.dma_start(
                                we_raw[:, :, :],
                                we_d[e, wh * WE_CH * 128 : (wh + 1) * WE_CH * 128, osl]
                                .rearrange("(kt p) o -> p kt o", p=128),
                            )
                            nc.vector.tensor_copy(we_t[:, ksl, :], we_raw[:, :, :])
                        for tt in range(TT_P):
                            ps = mm_psum.tile([128, ON], F32, tag="mmps")
                            for kt in range(KT):
                                nc.tensor.matmul(
                                    ps[:, :],
                                    xt_tiles[tt][:, kt, :],
                                    we_t[:, kt, :],
                                    start=(kt == 0), stop=(kt == KT - 1),
                                )
                            nc.vector.scalar_tensor_tensor(
                                out=acc_tiles[tt][:, osl],
                                in0=ps[:, :],
                                scalar=sw_tiles[tt][:, e : e + 1],
                                in1=acc_tiles[tt][:, osl],
                                op0=OP.mult,
                                op1=OP.add,
                            )
                            if e == E - 1:
                                nc.sync.dma_start(
                                    pred_d[t0 + tt * 128 : t0 + (tt + 1) * 128, osl],
                                    acc_tiles[tt][:, osl],
                                )
    nc.compile()
    return nc


_NC = None


def _get_nc():
    global _NC
    if _NC is None:
        _NC = build_nc()
    return _NC


TRACE = False
_LAST_RESULT = [None]


def make_in_maps(x, Wg, bg, We, be):
    x = np.ascontiguousarray(np.asarray(x, dtype=np.float32))
    Wg = np.ascontiguousarray(np.asarray(Wg, dtype=np.float32))
    bg = np.ascontiguousarray(np.asarray(bg, dtype=np.float32))
    We = np.ascontiguousarray(np.asarray(We, dtype=np.float32))
    be = np.ascontiguousarray(np.asarray(be, dtype=np.float32))
    ident = np.eye(128, dtype=np.float32)
    bgb = np.tile(bg.reshape(1, E), (128, 1)).astype(np.float32)
    return [
        {
            "x": x[i * B_L : (i + 1) * B_L],
            "Wg": Wg,
            "We": We,
            "be": be,
            "identity": ident,
            "bg_bcast": bgb,
        }
        for i in range(N_CORES)
    ]


def kernel(x, Wg, bg, We, be):
    nc = _get_nc()
    in_maps = make_in_maps(x, Wg, bg, We, be)
    res = bass_utils.run_bass_kernel_spmd(
        nc, in_maps, core_ids=list(range(N_CORES)), trace=TRACE
    )
    _LAST_RESULT[0] = res
    results = res.results

    pred = np.concatenate([r["pred"] for r in results], axis=0)
    logits = np.concatenate([r["logits"] for r in results], axis=0)
    sparse_weights = np.concatenate([r["sparse_weights"] for r in results], axis=0)
    top_indices = np.concatenate([r["top_indices"] for r in results], axis=0)
    top_values = np.concatenate([r["top_values"] for r in results], axis=0)
    return pred, logits, sparse_weights, top_indices, top_values
